# Initial kernel scaffold
#
"""Trainium2 Bass kernel for DrBCEncoder-style GNN message passing.

Strategy (8 NeuronCores, SPMD):
  - Nodes are dst-sharded: core c owns nodes [c*12500, (c+1)*12500), padded to
    12544 = 98*128 rows per core (total padded node space 100352 = 4*25088).
  - Each layer keeps the full node-major activation h [100352, 64] f32 in HBM
    (AllGather output); each core gathers rows h[src] for edges whose dst is in
    its shard via dma_gather (int16 indices -> 4 banks of 25088 rows).
  - segment-sum over edges is computed as a matmul: for each 128-edge chunk,
    PSUM[64 feat, 128 dst] += feats_chunk[128e, 64f].T @ onehot[128e, 128d]
    where onehot[e, d] = (dst_local[e] == d) * inv_deg[dst(e)] is built on the
    vector engines from an iota constant. This directly yields the transposed
    neighbor mean, which is the stationary operand for the neighbor-weight
    matmul.
  - Self path: h_tile [128, 64] is PE-transposed, then
    z[128n, 64f] = hT.T @ Ws_l.T + nmT.T @ Wn_l.T accumulated in PSUM.
  - LayerNorm over features runs on the free axis with per-partition stats,
    then relu + residual, store shard, AllGather for the next layer.

Host-side work is index preprocessing only (edge sort/bucketing, degree
bincount, layout packing, weight transposes).
"""
import sys

sys.path.insert(0, "/opt/trn_rl_repo")

import numpy as np

import concourse.bass as bass
import concourse.bacc as bacc
import concourse.tile as tile
from concourse import mybir
from concourse.bass_utils import run_bass_kernel_spmd

NCORES = 8
N_NODES = 100000
NODES_PER_CORE = 12500
PAD_PER_CORE = 12544            # 98 * 128
N_PAD = NCORES * PAD_PER_CORE   # 100352
TILES = PAD_PER_CORE // 128     # 98
BANKS = 4
BANK_ROWS = N_PAD // BANKS      # 25088 (< 32768 for int16 indices)
HID = 64
IN_DIM = 8
N_LAYERS = 3
LN_EPS = 1e-5

F32 = mybir.dt.float32
I16 = mybir.dt.int16
AOT = mybir.AluOpType
ACT_F = mybir.ActivationFunctionType

_program_cache = {}


def _remap(v):
    return (v // NODES_PER_CORE) * PAD_PER_CORE + (v % NODES_PER_CORE)


def _build_program(l_bank):
    """Build + compile the SPMD Bass program for a given per-bank slot size."""
    cb = l_bank // 128          # chunks per bank
    C = BANKS * cb              # chunks per tile
    lb16 = l_bank // 16

    nc = bacc.Bacc("TRN2", target_bir_lowering=False, debug=False,
                   num_devices=NCORES)

    idx_in = nc.dram_tensor("idx", [TILES, 128, BANKS * lb16], I16,
                            kind="ExternalInput")
    meta_in = nc.dram_tensor("meta", [TILES, 128, 2 * C], F32,
                             kind="ExternalInput")
    xt_in = nc.dram_tensor("xt", [IN_DIM, PAD_PER_CORE], F32,
                           kind="ExternalInput")
    w_in_t = nc.dram_tensor("w_in_t", [IN_DIM, HID], F32, kind="ExternalInput")
    ws_t = nc.dram_tensor("ws_t", [N_LAYERS, HID, HID], F32, kind="ExternalInput")
    wn_t = nc.dram_tensor("wn_t", [N_LAYERS, HID, HID], F32, kind="ExternalInput")
    bias_b = nc.dram_tensor("bias_b", [N_LAYERS, 128, HID], F32, kind="ExternalInput")
    gamma_b = nc.dram_tensor("gamma_b", [N_LAYERS, 128, HID], F32, kind="ExternalInput")
    beta_b = nc.dram_tensor("beta_b", [N_LAYERS, 128, HID], F32, kind="ExternalInput")
    b_in_b = nc.dram_tensor("b_in_b", [128, HID], F32, kind="ExternalInput")
    iota_in = nc.dram_tensor("iota", [128, 128], F32, kind="ExternalInput")
    ident_in = nc.dram_tensor("ident", [128, 128], F32, kind="ExternalInput")
    core_base_in = nc.dram_tensor("core_base", [1, 1], mybir.dt.int32,
                                  kind="ExternalInput")  # unused on device
    h_out = nc.dram_tensor("h_out", [PAD_PER_CORE, HID], F32,
                           kind="ExternalOutput")

    with tile.TileContext(nc) as tc:
        with (
            tc.tile_pool(name="const", bufs=1) as cp,
            tc.tile_pool(name="io", bufs=3) as iop,
            tc.tile_pool(name="feats", bufs=2) as fp,
            tc.tile_pool(name="oh", bufs=6) as ohp,
            tc.tile_pool(name="ln", bufs=3) as lnp,
            tc.tile_pool(name="ps_agg", bufs=2, space="PSUM") as ps_agg,
            tc.tile_pool(name="ps_tp", bufs=2, space="PSUM") as ps_tp,
            tc.tile_pool(name="ps_z", bufs=2, space="PSUM") as ps_z,
            tc.tile_pool(name="dram", bufs=1, space="DRAM") as dp,
        ):
            # ---- constants ----
            iota_t = cp.tile([128, 128], F32, tag="iota")
            nc.sync.dma_start(iota_t[:], iota_in[:])
            eps_t = cp.tile([128, 1], F32, tag="eps")
            nc.vector.memset(eps_t[:], LN_EPS)
            ident_t = cp.tile([128, 128], F32, tag="ident")
            nc.sync.dma_start(ident_t[:], ident_in[:])
            w_in_sb = cp.tile([IN_DIM, HID], F32, tag="w_in")
            nc.sync.dma_start(w_in_sb[:], w_in_t[:])
            b_in_sb = cp.tile([128, HID], F32, tag="b_in")
            nc.sync.dma_start(b_in_sb[:], b_in_b[:])
            ws_sb, wn_sb, bias_sb, gamma_sb, beta_sb = [], [], [], [], []
            for l in range(N_LAYERS):
                w1 = cp.tile([HID, HID], F32, tag=f"ws{l}")
                nc.sync.dma_start(w1[:], ws_t[l])
                ws_sb.append(w1)
                w2 = cp.tile([HID, HID], F32, tag=f"wn{l}")
                nc.sync.dma_start(w2[:], wn_t[l])
                wn_sb.append(w2)
                b1 = cp.tile([128, HID], F32, tag=f"bias{l}")
                nc.sync.dma_start(b1[:], bias_b[l])
                bias_sb.append(b1)
                g1 = cp.tile([128, HID], F32, tag=f"gamma{l}")
                nc.sync.dma_start(g1[:], gamma_b[l])
                gamma_sb.append(g1)
                be1 = cp.tile([128, HID], F32, tag=f"beta{l}")
                nc.sync.dma_start(be1[:], beta_b[l])
                beta_sb.append(be1)

            # ---- DRAM buffers ----
            h_bufs = [
                dp.tile([N_PAD, HID], F32, tag=f"h_buf{i}", name=f"h_buf{i}",
                        addr_space="Shared")
                for i in range(N_LAYERS)
            ]
            shards = [
                dp.tile([PAD_PER_CORE, HID], F32, tag=f"shard{i}",
                        name=f"shard{i}")
                for i in range(N_LAYERS)
            ]

            # ---- phase 0: h0 = relu(x @ W_in.T + b_in) for own shard ----
            for t in range(TILES):
                xt_sb = iop.tile([IN_DIM, 128], F32, tag="xt")
                nc.sync.dma_start(xt_sb[:], xt_in[:, t * 128:(t + 1) * 128])
                h0_ps = ps_z.tile([128, HID], F32, tag="z")
                nc.tensor.matmul(h0_ps[:], xt_sb[:], w_in_sb[:],
                                 start=True, stop=True)
                h0_sb = lnp.tile([128, HID], F32, tag="hnew")
                nc.vector.scalar_tensor_tensor(
                    h0_sb[:], h0_ps[:], 0.0, b_in_sb[:], AOT.bypass, AOT.add)
                h0r_sb = lnp.tile([128, HID], F32, tag="hnew2")
                nc.scalar.activation(h0r_sb[:], h0_sb[:], ACT_F.Relu)
                nc.sync.dma_start(shards[0][t * 128:(t + 1) * 128, :], h0r_sb[:])
            nc.gpsimd.collective_compute(
                "AllGather", AOT.bypass,
                ins=[shards[0].opt()], outs=[h_bufs[0].opt()],
                replica_groups=[list(range(NCORES))])

            # ---- layers ----
            for l in range(N_LAYERS):
                src_buf = h_bufs[l]
                own_shard = shards[l]
                for t in range(TILES):
                    idx_t = iop.tile([128, BANKS * lb16], I16, tag="idx")
                    nc.sync.dma_start(idx_t[:], idx_in[t])
                    meta_t = iop.tile([128, 2 * C], F32, tag="meta")
                    nc.sync.dma_start(meta_t[:], meta_in[t])

                    feats = fp.tile([128, C, HID], F32, tag="feats")
                    for b in range(BANKS):
                        nc.gpsimd.dma_gather(
                            feats[:, b * cb:(b + 1) * cb, :],
                            src_buf[b * BANK_ROWS:(b + 1) * BANK_ROWS, :],
                            idx_t[:, b * lb16:(b + 1) * lb16],
                            l_bank, l_bank, HID,
                            single_packet=(l_bank <= 1024))

                    agg = ps_agg.tile([HID, 128], F32, tag="agg")
                    for k in range(C):
                        oh = ohp.tile([128, 128], F32, tag="oh")
                        nc.any.tensor_scalar(
                            oh[:], iota_t[:],
                            meta_t[:, k:k + 1], meta_t[:, C + k:C + k + 1],
                            AOT.is_equal, AOT.mult)
                        nc.tensor.matmul(agg[:], feats[:, k, :], oh[:],
                                         start=(k == 0), stop=(k == C - 1))

                    nmT = lnp.tile([HID, 128], F32, tag="nmT")
                    nc.vector.tensor_copy(nmT[:], agg[:])

                    h_t = iop.tile([128, HID], F32, tag="h_t")
                    nc.sync.dma_start(
                        h_t[:], own_shard[t * 128:(t + 1) * 128, :])
                    tp_ps = ps_tp.tile([HID, 128], F32, tag="tp")
                    nc.tensor.transpose(tp_ps[:], h_t[:], ident_t[:])
                    hT_t = lnp.tile([HID, 128], F32, tag="hT")
                    nc.vector.tensor_copy(hT_t[:], tp_ps[:])

                    z_ps = ps_z.tile([128, HID], F32, tag="z")
                    nc.tensor.matmul(z_ps[:], hT_t[:], ws_sb[l][:],
                                     start=True, stop=False)
                    nc.tensor.matmul(z_ps[:], nmT[:], wn_sb[l][:],
                                     start=False, stop=True)

                    # LayerNorm + affine + relu + residual
                    stats = lnp.tile([128, 4], F32, tag="stats")
                    zb = lnp.tile([128, HID], F32, tag="zb")
                    nc.vector.scalar_tensor_tensor(
                        zb[:], z_ps[:], 0.0, bias_sb[l][:],
                        AOT.bypass, AOT.add, accum_out=stats[:, 0:1])
                    zsq = lnp.tile([128, HID], F32, tag="zsq")
                    nc.scalar.activation(zsq[:], zb[:], ACT_F.Square,
                                         accum_out=stats[:, 1:2])
                    mstat = lnp.tile([128, 2], F32, tag="mstat")
                    nc.vector.tensor_scalar(
                        mstat[:], stats[:, 0:2], 1.0 / HID, None, AOT.mult)
                    m2 = lnp.tile([128, 1], F32, tag="m2")
                    nc.vector.tensor_tensor(
                        m2[:], mstat[:, 0:1], mstat[:, 0:1], AOT.mult)
                    var = lnp.tile([128, 1], F32, tag="var")
                    nc.vector.tensor_tensor(
                        var[:], mstat[:, 1:2], m2[:], AOT.subtract)
                    std = lnp.tile([128, 1], F32, tag="std")
                    nc.scalar.activation(std[:], var[:], ACT_F.Sqrt,
                                         bias=eps_t[:])
                    rstd = lnp.tile([128, 1], F32, tag="rstd")
                    nc.vector.reciprocal(rstd[:], std[:])
                    t2 = lnp.tile([128, HID], F32, tag="t2")
                    nc.vector.tensor_scalar(
                        t2[:], zb[:], mstat[:, 0:1], rstd[:],
                        AOT.subtract, AOT.mult)
                    t3 = lnp.tile([128, HID], F32, tag="t3")
                    nc.vector.scalar_tensor_tensor(
                        t3[:], t2[:], 0.0, gamma_sb[l][:], AOT.bypass, AOT.mult)
                    t4 = lnp.tile([128, HID], F32, tag="t4")
                    nc.vector.scalar_tensor_tensor(
                        t4[:], t3[:], 0.0, beta_sb[l][:], AOT.bypass, AOT.add)
                    h_new = lnp.tile([128, HID], F32, tag="hnew")
                    nc.vector.scalar_tensor_tensor(
                        h_new[:], t4[:], 0.0, h_t[:], AOT.max, AOT.add)

                    if l == N_LAYERS - 1:
                        nc.sync.dma_start(
                            h_out[t * 128:(t + 1) * 128, :], h_new[:])
                    else:
                        nc.sync.dma_start(
                            shards[l + 1][t * 128:(t + 1) * 128, :], h_new[:])
                if l < N_LAYERS - 1:
                    nc.gpsimd.collective_compute(
                        "AllGather", AOT.bypass,
                        ins=[shards[l + 1].opt()],
                        outs=[h_bufs[l + 1].opt()],
                        replica_groups=[list(range(NCORES))])

    nc.compile()
    return nc


def _preprocess(x, edge_src, edge_dst, W_in, b_in, Ws_self, Ws_neigh,
                biases, gammas, betas):
    """Pure index/layout preprocessing on the host. Returns (in_maps, l_bank)."""
    src = edge_src.astype(np.int64)
    dst = edge_dst.astype(np.int64)
    rsrc = _remap(src)
    rdst = _remap(dst)

    tile_g = rdst // 128              # global tile id in padded space, 0..783
    dst_loc = (rdst % 128).astype(np.float32)
    bank = rsrc // BANK_ROWS
    idx_loc = (rsrc - bank * BANK_ROWS).astype(np.int16)

    deg = np.bincount(dst, minlength=N_NODES)
    invdeg = np.where(deg > 0, 1.0 / np.maximum(deg, 1), 0.0).astype(np.float32)
    inv_e = invdeg[dst]

    n_groups = NCORES * TILES * BANKS
    key = tile_g * BANKS + bank
    order = np.argsort(key, kind="stable")
    key_s = key[order]
    counts = np.bincount(key_s, minlength=n_groups)
    l_bank = max(128, int(np.ceil(counts.max() / 128)) * 128)
    cb = l_bank // 128
    C = BANKS * cb
    lb16 = l_bank // 16

    starts = np.zeros(n_groups, dtype=np.int64)
    starts[1:] = np.cumsum(counts)[:-1]
    rank = np.arange(len(src)) - starts[key_s]
    pos = key_s * l_bank + rank       # global padded position

    total = n_groups * l_bank
    idx_full = np.zeros(total, dtype=np.int16)
    idx_full[pos] = idx_loc[order]
    dstl_full = np.full(total, -1.0, dtype=np.float32)
    dstl_full[pos] = dst_loc[order]
    inv_full = np.zeros(total, dtype=np.float32)
    inv_full[pos] = inv_e[order]

    # idx: [784, BANKS, l_bank] -> wrap16 -> replicate to 128 partitions
    idx_w = idx_full.reshape(NCORES * TILES, BANKS, lb16, 16)
    idx_w = idx_w.transpose(0, 1, 3, 2)                    # [784, B, 16, lb16]
    idx_w = np.broadcast_to(idx_w[:, :, None, :, :],
                            (NCORES * TILES, BANKS, 8, 16, lb16))
    idx_w = idx_w.transpose(0, 2, 3, 1, 4).reshape(
        NCORES, TILES, 128, BANKS * lb16).copy()

    # meta: positions within a tile wrap mod 128 across all banks' chunks
    dstl_w = dstl_full.reshape(NCORES * TILES, C, 128).transpose(0, 2, 1)
    inv_w = inv_full.reshape(NCORES * TILES, C, 128).transpose(0, 2, 1)
    meta = np.concatenate([dstl_w, inv_w], axis=2).reshape(
        NCORES, TILES, 128, 2 * C).astype(np.float32).copy()

    # xT per core
    xp = np.zeros((N_PAD, IN_DIM), dtype=np.float32)
    xp[_remap(np.arange(N_NODES))] = x
    xp = xp.reshape(NCORES, PAD_PER_CORE, IN_DIM)

    w_in_t = np.ascontiguousarray(W_in.T.astype(np.float32))
    ws_t = np.ascontiguousarray(Ws_self.transpose(0, 2, 1).astype(np.float32))
    wn_t = np.ascontiguousarray(Ws_neigh.transpose(0, 2, 1).astype(np.float32))
    bias_b = np.ascontiguousarray(
        np.broadcast_to(biases[:, None, :], (N_LAYERS, 128, HID)).astype(np.float32))
    gamma_b = np.ascontiguousarray(
        np.broadcast_to(gammas[:, None, :], (N_LAYERS, 128, HID)).astype(np.float32))
    beta_b = np.ascontiguousarray(
        np.broadcast_to(betas[:, None, :], (N_LAYERS, 128, HID)).astype(np.float32))
    b_in_bc = np.ascontiguousarray(
        np.broadcast_to(b_in[None, :], (128, HID)).astype(np.float32))
    iota = np.tile(np.arange(128, dtype=np.float32), (128, 1))
    ident = np.eye(128, dtype=np.float32)

    in_maps = []
    for c in range(NCORES):
        in_maps.append({
            "idx": idx_w[c],
            "meta": meta[c],
            "xt": np.ascontiguousarray(xp[c].T),
            "w_in_t": w_in_t,
            "ws_t": ws_t,
            "wn_t": wn_t,
            "bias_b": bias_b,
            "gamma_b": gamma_b,
            "beta_b": beta_b,
            "b_in_b": b_in_bc,
            "iota": iota,
            "ident": ident,
            "core_base": np.array([[c]], dtype=np.int32),
        })
    return in_maps, l_bank


def kernel(**inputs):
    in_maps, l_bank = _preprocess(
        np.asarray(inputs["x"]), np.asarray(inputs["edge_src"]),
        np.asarray(inputs["edge_dst"]), np.asarray(inputs["W_in"]),
        np.asarray(inputs["b_in"]), np.asarray(inputs["Ws_self"]),
        np.asarray(inputs["Ws_neigh"]), np.asarray(inputs["biases"]),
        np.asarray(inputs["gammas"]), np.asarray(inputs["betas"]))

    if l_bank not in _program_cache:
        _program_cache[l_bank] = _build_program(l_bank)
    nc = _program_cache[l_bank]

    res = run_bass_kernel_spmd(nc, in_maps, list(range(NCORES)))
    out = np.concatenate(
        [res.results[c]["h_out"][:NODES_PER_CORE] for c in range(NCORES)],
        axis=0)
    return out.astype(np.float32)



# revision 4
# speedup vs baseline: 1.0024x; 1.0024x over previous
"""Trainium2 Bass kernel v2 for DrBCEncoder GNN message passing.

vs v1 baseline:
  - h activations bf16, PACKED: DRAM layout [superrow, 128] bf16 where
    superrow r holds nodes 2r (cols 0:64) and 2r+1 (cols 64:128) -> 256B
    gather elements with zero padding waste; AllGather bytes halved.
  - node order inside a tile is q-order: q = (local//2) + 64*(local%2)
    (host permutes inputs/outputs; onehot dst columns use q).
  - gather groups: per tile 4 edge groups (bank b in {0,1} x parity p);
    one dma_gather per (tile-group, bank) covering both parities ->
    ~TG x fewer Q7 calls (dma_gather has ~8.5us fixed cost each).
  - one-hot matrices precomputed on host (bf16, inv_deg folded), DMA'd
    per tile; optional fraction built on DVE (OH_DVE_FRAC knob).
  - PSUM->SBUF copies routed to the Scalar (ACT) engine.
"""
import sys

sys.path.insert(0, "/opt/trn_rl_repo")

import numpy as np
import ml_dtypes

import concourse.bass as bass
import concourse.bacc as bacc
import concourse.tile as tile
from concourse import mybir
from concourse.bass_utils import run_bass_kernel_spmd

NCORES = 8
N_NODES = 100000
NODES_PER_CORE = 12500
PAD_PER_CORE = 12544            # 98 * 128
N_PAD = NCORES * PAD_PER_CORE   # 100352
TILES = PAD_PER_CORE // 128     # 98
SR = N_PAD // 2                 # 50176 superrows
SR_CORE = PAD_PER_CORE // 2     # 6272
BANKS = 2
BANK_ROWS = SR // BANKS         # 25088 superrows per bank
HID = 64
IN_DIM = 8
N_LAYERS = 3
LN_EPS = 1e-5

GATHER_TG = 2                   # tiles per gather call (K = TG*2*l_bank)
OH_DVE_FRAC = 0.0               # fraction of onehot chunks built on DVE

F32 = mybir.dt.float32
BF16 = mybir.dt.bfloat16
I16 = mybir.dt.int16
AOT = mybir.AluOpType
ACT_F = mybir.ActivationFunctionType

_program_cache = {}


def _remap(v):
    return (v // NODES_PER_CORE) * PAD_PER_CORE + (v % NODES_PER_CORE)


def _groups(tg):
    gs = []
    t0 = 0
    while t0 < TILES:
        gs.append(list(range(t0, min(t0 + tg, TILES))))
        t0 += tg
    return gs


def _oh_on_dve(t, g, j, cb):
    if OH_DVE_FRAC <= 0.0:
        return False
    k = (t * 4 + g) * cb + j
    return (k * 2654435761 % 1000) < OH_DVE_FRAC * 1000


def _build_program(l_bank):
    cb = l_bank // 128
    C = 4 * cb                  # chunks per tile (2 banks x 2 parities)
    groups = _groups(GATHER_TG)

    nc = bacc.Bacc("TRN2", target_bir_lowering=False, debug=False,
                   num_devices=NCORES)

    tot16 = TILES * C * 8
    idx_in = nc.dram_tensor("idx", [128, tot16], I16, kind="ExternalInput")
    oh_in = nc.dram_tensor("oh", [TILES, 128, C * 128], BF16,
                           kind="ExternalInput")
    meta_in = nc.dram_tensor("meta", [TILES, 128, 2 * C], F32,
                             kind="ExternalInput")
    xt_in = nc.dram_tensor("xt", [IN_DIM, PAD_PER_CORE], BF16,
                           kind="ExternalInput")
    w_in_t = nc.dram_tensor("w_in_t", [IN_DIM, HID], BF16, kind="ExternalInput")
    ws_t = nc.dram_tensor("ws_t", [N_LAYERS, HID, HID], BF16, kind="ExternalInput")
    wn_t = nc.dram_tensor("wn_t", [N_LAYERS, HID, HID], BF16, kind="ExternalInput")
    bias_b = nc.dram_tensor("bias_b", [N_LAYERS, 128, HID], F32, kind="ExternalInput")
    gamma_b = nc.dram_tensor("gamma_b", [N_LAYERS, 128, HID], F32, kind="ExternalInput")
    beta_b = nc.dram_tensor("beta_b", [N_LAYERS, 128, HID], F32, kind="ExternalInput")
    b_in_b = nc.dram_tensor("b_in_b", [128, HID], F32, kind="ExternalInput")
    iota_in = nc.dram_tensor("iota", [128, 128], BF16, kind="ExternalInput")
    ident_in = nc.dram_tensor("ident", [128, 128], BF16, kind="ExternalInput")
    h_out = nc.dram_tensor("h_out", [PAD_PER_CORE, HID], F32,
                           kind="ExternalOutput")

    with tile.TileContext(nc) as tc:
        with (
            tc.tile_pool(name="const", bufs=1) as cp,
            tc.tile_pool(name="io", bufs=4) as iop,
            tc.tile_pool(name="idxp", bufs=4) as idxp,
            tc.tile_pool(name="feats", bufs=3) as fp,
            tc.tile_pool(name="ohp", bufs=3) as ohp,
            tc.tile_pool(name="ohv", bufs=4) as ohv,
            tc.tile_pool(name="ln", bufs=4) as lnp,
            tc.tile_pool(name="ps_agg", bufs=3, space="PSUM") as ps_agg,
            tc.tile_pool(name="ps_tp", bufs=2, space="PSUM") as ps_tp,
            tc.tile_pool(name="ps_z", bufs=3, space="PSUM") as ps_z,
            tc.tile_pool(name="dram", bufs=1, space="DRAM") as dp,
        ):
            # ---- constants ----
            iota_t = cp.tile([128, 128], BF16, tag="iota")
            nc.sync.dma_start(iota_t[:], iota_in[:])
            eps_t = cp.tile([128, 1], F32, tag="eps")
            nc.vector.memset(eps_t[:], LN_EPS)
            ident_t = cp.tile([128, 128], BF16, tag="ident")
            nc.sync.dma_start(ident_t[:], ident_in[:])
            w_in_sb = cp.tile([IN_DIM, HID], BF16, tag="w_in")
            nc.sync.dma_start(w_in_sb[:], w_in_t[:])
            b_in_sb = cp.tile([128, HID], F32, tag="b_in")
            nc.sync.dma_start(b_in_sb[:], b_in_b[:])
            ws_sb, wn_sb, bias_sb, gamma_sb, beta_sb = [], [], [], [], []
            for l in range(N_LAYERS):
                w1 = cp.tile([HID, HID], BF16, tag=f"ws{l}")
                nc.sync.dma_start(w1[:], ws_t[l])
                ws_sb.append(w1)
                w2 = cp.tile([HID, HID], BF16, tag=f"wn{l}")
                nc.sync.dma_start(w2[:], wn_t[l])
                wn_sb.append(w2)
                b1 = cp.tile([128, HID], F32, tag=f"bias{l}")
                nc.sync.dma_start(b1[:], bias_b[l])
                bias_sb.append(b1)
                g1 = cp.tile([128, HID], F32, tag=f"gamma{l}")
                nc.sync.dma_start(g1[:], gamma_b[l])
                gamma_sb.append(g1)
                be1 = cp.tile([128, HID], F32, tag=f"beta{l}")
                nc.sync.dma_start(be1[:], beta_b[l])
                beta_sb.append(be1)

            # ---- DRAM buffers (packed superrows) ----
            h_bufs = [
                dp.tile([SR, 128], BF16, tag=f"h_buf{i}", name=f"h_buf{i}",
                        addr_space="Shared")
                for i in range(N_LAYERS)
            ]
            shards = [
                dp.tile([SR_CORE, 128], BF16, tag=f"shard{i}",
                        name=f"shard{i}")
                for i in range(N_LAYERS)
            ]

            def store_packed(dst_shard, t, src_tile):
                # src_tile [128(q), 64] -> packed superrows
                nc.sync.dma_start(
                    dst_shard[t * 64:(t + 1) * 64, 0:HID], src_tile[0:64, :])
                nc.sync.dma_start(
                    dst_shard[t * 64:(t + 1) * 64, HID:128], src_tile[64:128, :])

            # ---- phase 0: h0 = relu(x @ W_in.T + b_in), q-order ----
            for t in range(TILES):
                xt_sb = iop.tile([IN_DIM, 128], BF16, tag="xt")
                nc.sync.dma_start(xt_sb[:], xt_in[:, t * 128:(t + 1) * 128])
                h0_ps = ps_z.tile([128, HID], F32, tag="z")
                nc.tensor.matmul(h0_ps[:], xt_sb[:], w_in_sb[:],
                                 start=True, stop=True)
                h0_sb = lnp.tile([128, HID], F32, tag="hnew32")
                nc.vector.scalar_tensor_tensor(
                    h0_sb[:], h0_ps[:], 0.0, b_in_sb[:], AOT.bypass, AOT.add)
                h0r_sb = lnp.tile([128, HID], BF16, tag="hnew")
                nc.scalar.activation(h0r_sb[:], h0_sb[:], ACT_F.Relu)
                store_packed(shards[0], t, h0r_sb)
            nc.gpsimd.collective_compute(
                "AllGather", AOT.bypass,
                ins=[shards[0].opt()], outs=[h_bufs[0].opt()],
                replica_groups=[list(range(NCORES))])

            # ---- layers ----
            dma_sem = nc.alloc_semaphore("gather_dma")
            for l in range(N_LAYERS):
                src_buf = h_bufs[l]
                own_shard = shards[l]
                for gi, tlist in enumerate(groups):
                    ng = len(tlist)
                    feats_b = []
                    for b in range(BANKS):
                        K = ng * 2 * l_bank
                        # chunk-stream offset of this call
                        off16 = (tlist[0] * C + b * ng * 2 * cb) * 8
                        idx_t = idxp.tile([128, K // 16], I16, tag=f"idx{b}")
                        nc.sync.dma_start(
                            idx_t[:], idx_in[:, off16:off16 + K // 16])
                        ft = fp.tile([128, ng * 2 * cb, 128], BF16,
                                     tag=f"feats{b}")
                        # prep-only: Q7 emits descriptors, engine is NOT
                        # held for the transfer; trigger fires the DMA and
                        # consumers wait on the completion sem (Tile-managed)
                        nc.gpsimd.dma_gather(
                            ft[:],
                            src_buf[b * BANK_ROWS:(b + 1) * BANK_ROWS, :],
                            idx_t[:], K, K, 128,
                            prepare_only=True, sem=dma_sem,
                            single_packet=False)
                        nc.gpsimd.trigger_dma(count=None)
                        feats_b.append(ft)

                    for ti, t in enumerate(tlist):
                        ohd = ohp.tile([128, C * 128], BF16, tag="ohd")
                        nc.sync.dma_start(ohd[:], oh_in[t])
                        need_meta = any(
                            _oh_on_dve(t, g, j, cb)
                            for g in range(4) for j in range(cb))
                        if need_meta:
                            meta_t = iop.tile([128, 2 * C], F32, tag="meta")
                            nc.sync.dma_start(meta_t[:], meta_in[t])

                        agg = ps_agg.tile([HID, 128], F32, tag="agg")
                        kk = 0
                        for b in range(BANKS):
                            for p in range(2):
                                g = b * 2 + p
                                for j in range(cb):
                                    c_i = g * cb + j      # tile-major chunk
                                    if _oh_on_dve(t, g, j, cb):
                                        ohc = ohv.tile([128, 128], BF16,
                                                       tag="ohc")
                                        nc.vector.tensor_scalar(
                                            ohc[:], iota_t[:],
                                            meta_t[:, c_i:c_i + 1],
                                            meta_t[:, C + c_i:C + c_i + 1],
                                            AOT.is_equal, AOT.mult)
                                        mov = ohc[:]
                                    else:
                                        mov = ohd[:, c_i * 128:(c_i + 1) * 128]
                                    pos = (ti * 2 + p) * cb + j
                                    nc.tensor.matmul(
                                        agg[:],
                                        feats_b[b][:, pos,
                                                   p * HID:(p + 1) * HID],
                                        mov,
                                        start=(kk == 0), stop=(kk == C - 1))
                                    kk += 1

                        nmT = lnp.tile([HID, 128], BF16, tag="nmT")
                        nc.scalar.activation(nmT[:], agg[:], ACT_F.Copy)

                        # self path: load packed rows, unpack to q-order
                        h_t = iop.tile([128, HID], BF16, tag="h_t")
                        nc.sync.dma_start(
                            h_t[0:64, :],
                            own_shard[t * 64:(t + 1) * 64, 0:HID])
                        nc.sync.dma_start(
                            h_t[64:128, :],
                            own_shard[t * 64:(t + 1) * 64, HID:128])
                        tp_ps = ps_tp.tile([HID, 128], BF16, tag="tp")
                        nc.tensor.transpose(tp_ps[:], h_t[:], ident_t[:])
                        hT_t = lnp.tile([HID, 128], BF16, tag="hT")
                        nc.scalar.activation(hT_t[:], tp_ps[:], ACT_F.Copy)

                        z_ps = ps_z.tile([128, HID], F32, tag="z")
                        nc.tensor.matmul(z_ps[:], hT_t[:], ws_sb[l][:],
                                         start=True, stop=False)
                        nc.tensor.matmul(z_ps[:], nmT[:], wn_sb[l][:],
                                         start=False, stop=True)

                        # LayerNorm + affine + relu + residual
                        stats = lnp.tile([128, 4], F32, tag="stats")
                        zb = lnp.tile([128, HID], F32, tag="zb")
                        nc.vector.scalar_tensor_tensor(
                            zb[:], z_ps[:], 0.0, bias_sb[l][:],
                            AOT.bypass, AOT.add)
                        zsq = lnp.tile([128, HID], F32, tag="zsq")
                        nc.scalar.activation(zsq[:], zb[:], ACT_F.Square)
                        nc.vector.tensor_reduce(
                            stats[:, 0:1], zb[:], mybir.AxisListType.X, AOT.add)
                        nc.vector.tensor_reduce(
                            stats[:, 1:2], zsq[:], mybir.AxisListType.X, AOT.add)
                        # mean/meansq scaling on ACT (a tiny-FD DVE
                        # tensor_scalar here measured 4.6-19.7us under load)
                        mu = lnp.tile([128, 1], F32, tag="mu")
                        nc.scalar.activation(mu[:], stats[:, 0:1], ACT_F.Copy,
                                             scale=1.0 / HID)
                        msq = lnp.tile([128, 1], F32, tag="msq")
                        nc.scalar.activation(msq[:], stats[:, 1:2], ACT_F.Copy,
                                             scale=1.0 / HID)
                        m2 = lnp.tile([128, 1], F32, tag="m2")
                        nc.scalar.activation(m2[:], mu[:], ACT_F.Square)
                        var = lnp.tile([128, 1], F32, tag="var")
                        nc.vector.tensor_tensor(
                            var[:], msq[:], m2[:], AOT.subtract)
                        std = lnp.tile([128, 1], F32, tag="std")
                        nc.scalar.activation(std[:], var[:], ACT_F.Sqrt,
                                             bias=eps_t[:])
                        rstd = lnp.tile([128, 1], F32, tag="rstd")
                        nc.vector.reciprocal(rstd[:], std[:])
                        t2 = lnp.tile([128, HID], F32, tag="t2")
                        nc.vector.tensor_scalar(
                            t2[:], zb[:], mu[:], rstd[:],
                            AOT.subtract, AOT.mult)
                        t3 = lnp.tile([128, HID], F32, tag="t3")
                        nc.vector.scalar_tensor_tensor(
                            t3[:], t2[:], 0.0, gamma_sb[l][:], AOT.bypass,
                            AOT.mult)
                        t4 = lnp.tile([128, HID], F32, tag="t4")
                        nc.vector.scalar_tensor_tensor(
                            t4[:], t3[:], 0.0, beta_sb[l][:], AOT.bypass,
                            AOT.add)
                        if l == N_LAYERS - 1:
                            h_new = lnp.tile([128, HID], F32, tag="hnew32")
                            nc.vector.scalar_tensor_tensor(
                                h_new[:], t4[:], 0.0, h_t[:],
                                AOT.max, AOT.add)
                            nc.sync.dma_start(
                                h_out[t * 128:(t + 1) * 128, :], h_new[:])
                        else:
                            h_new = lnp.tile([128, HID], BF16, tag="hnew")
                            nc.vector.scalar_tensor_tensor(
                                h_new[:], t4[:], 0.0, h_t[:],
                                AOT.max, AOT.add)
                            store_packed(shards[l + 1], t, h_new)
                if l < N_LAYERS - 1:
                    nc.gpsimd.collective_compute(
                        "AllGather", AOT.bypass,
                        ins=[shards[l + 1].opt()],
                        outs=[h_bufs[l + 1].opt()],
                        replica_groups=[list(range(NCORES))])

    _rewire_prep_sems(nc)
    nc.compile()
    return nc


def _rewire_prep_sems(nc):
    """Point each gather-prep's descriptor-completion sem (on_update[0]) at
    the DMASW lane sem Tile scheduled it on. Tile emits consumer waits on
    the lane sems (+16 per prep) but leaves the user sem in on_update[0];
    hardware bakes exactly on_update[0] into the descriptors, so without
    this the lane sems never advance and consumers race the DMA."""
    from concourse.tile_scheduler import PROC_NAMES
    lane_sem_id = {}
    for sid, names in nc.m.ant_sem_names.items():
        for n in names:
            base = n.rsplit("_", 1)[0]
            if base.startswith("DMASW"):
                lane_sem_id[base] = int(sid)
    n_fixed = 0
    lane_ids = set(lane_sem_id.values())
    for f in nc.m.functions:
        for bb in f.blocks:
            for ins in bb.instructions:
                if (type(ins).__name__ == "InstDMAGatherAnt"
                        and getattr(ins, "gen_mode", 0) == 1):
                    proc = ins.bass_scheduled_proc
                    pname = PROC_NAMES[proc]
                    assert pname.startswith("DMASW"), pname
                    upd = ins.sync_info.on_update
                    assert upd and upd[0].update_value == 16, upd
                    upd[0].id = lane_sem_id[pname]
                    n_fixed += 1
    assert n_fixed > 0
    # The prep's engine completion ALSO fires on_update[0] (+16) in addition
    # to the descriptor-baked +16 at DMA completion, so each prep advances
    # its lane sem by 32 total. Double the consumer wait thresholds so a
    # wait requires BOTH the engine fire and the DMA completion. Sound
    # because the descriptor ring (< 2 preps of 4608 descs) keeps the
    # engine from running a full 8-lane cycle ahead of the drain.
    for f in nc.m.functions:
        for bb in f.blocks:
            for ins in bb.instructions:
                si = ins.sync_info
                if not si:
                    continue
                for w in (si.on_wait or []):
                    if w.id in lane_ids:
                        w.wait_value = 2 * w.wait_value


def _preprocess(x, edge_src, edge_dst, W_in, b_in, Ws_self, Ws_neigh,
                biases, gammas, betas):
    src = edge_src.astype(np.int64)
    dst = edge_dst.astype(np.int64)
    rsrc = _remap(src)
    rdst = _remap(dst)

    core = rdst // PAD_PER_CORE
    tl = (rdst % PAD_PER_CORE) // 128
    vloc = rdst % 128
    # q-order position within tile
    qloc = (vloc // 2 + 64 * (vloc % 2)).astype(np.float32)

    sr = rsrc // 2
    parity = (rsrc % 2).astype(np.int64)
    bank = sr // BANK_ROWS
    idx_loc = (sr - bank * BANK_ROWS).astype(np.int16)

    deg = np.bincount(dst, minlength=N_NODES)
    invdeg = np.where(deg > 0, 1.0 / np.maximum(deg, 1), 0.0).astype(np.float32)
    inv_e = invdeg[dst]

    # group = (core, tl, b, p); tile-major chunk group g = b*2+p
    n_groups = NCORES * TILES * 4
    key = ((core * TILES + tl) * 2 + bank) * 2 + parity
    # within each group, order slots by ascending source address for
    # better HBM row-buffer locality during the gather drain
    order = np.lexsort((idx_loc, key))
    key_s = key[order]
    counts = np.bincount(key_s, minlength=n_groups)
    l_bank = max(128, int(np.ceil(counts.max() / 128)) * 128)
    cb = l_bank // 128
    C = 4 * cb

    starts = np.zeros(n_groups, dtype=np.int64)
    starts[1:] = np.cumsum(counts)[:-1]
    rank = np.arange(len(src)) - starts[key_s]
    slot_in_core = (key_s % (TILES * 4)) * l_bank + rank
    core_s = key_s // (TILES * 4)

    per_core = TILES * 4 * l_bank
    idx_full = np.zeros((NCORES, per_core), dtype=np.int16)
    dstl_full = np.full((NCORES, per_core), -1.0, dtype=np.float32)
    inv_full = np.zeros((NCORES, per_core), dtype=np.float32)
    idx_full[core_s, slot_in_core] = idx_loc[order]
    dstl_full[core_s, slot_in_core] = qloc[order]
    inv_full[core_s, slot_in_core] = inv_e[order]

    # tile-major chunk id for (t, g=b*2+p, j) = (t*4+g)*cb + j
    # gather call-major order: for G: for b: for t in G: for p: for j
    groups = _groups(GATHER_TG)
    perm = np.empty(TILES * C, dtype=np.int64)
    pos = 0
    for tlist in groups:
        for b in range(BANKS):
            for t in tlist:
                for p in range(2):
                    for j in range(cb):
                        perm[pos] = (t * 4 + b * 2 + p) * cb + j
                        pos += 1
    idx_tm = idx_full.reshape(NCORES, TILES * C, 128)
    dstl_tm = dstl_full.reshape(NCORES, TILES * C, 128)
    inv_tm = inv_full.reshape(NCORES, TILES * C, 128)
    idx_cm = idx_tm[:, perm, :]

    tot = TILES * C * 128
    idx_w = idx_cm.reshape(NCORES, tot // 16, 16).transpose(0, 2, 1)
    idx_w = np.broadcast_to(idx_w[:, None, :, :], (NCORES, 8, 16, tot // 16))
    idx_w = np.ascontiguousarray(idx_w.reshape(NCORES, 128, tot // 16))

    # onehot [TILES, 128e, C*128d] bf16, tile-major chunks
    oh = np.zeros((NCORES, TILES * C, 128, 128), dtype=ml_dtypes.bfloat16)
    for c in range(NCORES):
        cc, ee = np.nonzero(dstl_tm[c] >= 0)
        dd = dstl_tm[c][cc, ee].astype(np.int64)
        oh[c][cc, ee, dd] = inv_tm[c][cc, ee].astype(ml_dtypes.bfloat16)
    oh = np.ascontiguousarray(
        oh.reshape(NCORES, TILES, C, 128, 128).transpose(0, 1, 3, 2, 4)
        .reshape(NCORES, TILES, 128, C * 128))

    # meta (tile-major) for DVE-built chunks
    dstl_w = dstl_tm.reshape(NCORES, TILES, C, 128).transpose(0, 1, 3, 2)
    inv_w = inv_tm.reshape(NCORES, TILES, C, 128).transpose(0, 1, 3, 2)
    meta = np.ascontiguousarray(
        np.concatenate([dstl_w, inv_w], axis=3)).astype(np.float32)

    # x permuted to q-order, transposed per core
    qperm = np.arange(N_PAD)
    t_all = qperm // 128
    v_all = qperm % 128
    node_of_q = t_all * 128 + (v_all % 64) * 2 + (v_all // 64)
    xp = np.zeros((N_PAD, IN_DIM), dtype=np.float32)
    xp[_remap(np.arange(N_NODES))] = x
    xq = xp[node_of_q]                       # row i = x of node at q-position i
    xq = xq.reshape(NCORES, PAD_PER_CORE, IN_DIM)

    bf = ml_dtypes.bfloat16
    w_in_t = np.ascontiguousarray(W_in.T).astype(bf)
    ws_t = np.ascontiguousarray(Ws_self.transpose(0, 2, 1)).astype(bf)
    wn_t = np.ascontiguousarray(Ws_neigh.transpose(0, 2, 1)).astype(bf)
    bias_b = np.ascontiguousarray(
        np.broadcast_to(biases[:, None, :], (N_LAYERS, 128, HID))).astype(np.float32)
    gamma_b = np.ascontiguousarray(
        np.broadcast_to(gammas[:, None, :], (N_LAYERS, 128, HID))).astype(np.float32)
    beta_b = np.ascontiguousarray(
        np.broadcast_to(betas[:, None, :], (N_LAYERS, 128, HID))).astype(np.float32)
    b_in_bc = np.ascontiguousarray(
        np.broadcast_to(b_in[None, :], (128, HID))).astype(np.float32)
    iota = np.tile(np.arange(128), (128, 1)).astype(bf)
    ident = np.eye(128).astype(bf)

    in_maps = []
    for c in range(NCORES):
        in_maps.append({
            "idx": idx_w[c],
            "oh": oh[c],
            "meta": meta[c],
            "xt": np.ascontiguousarray(xq[c].T).astype(bf),
            "w_in_t": w_in_t,
            "ws_t": ws_t,
            "wn_t": wn_t,
            "bias_b": bias_b,
            "gamma_b": gamma_b,
            "beta_b": beta_b,
            "b_in_b": b_in_bc,
            "iota": iota,
            "ident": ident,
        })
    return in_maps, l_bank, node_of_q


def kernel(**inputs):
    in_maps, l_bank, node_of_q = _preprocess(
        np.asarray(inputs["x"]), np.asarray(inputs["edge_src"]),
        np.asarray(inputs["edge_dst"]), np.asarray(inputs["W_in"]),
        np.asarray(inputs["b_in"]), np.asarray(inputs["Ws_self"]),
        np.asarray(inputs["Ws_neigh"]), np.asarray(inputs["biases"]),
        np.asarray(inputs["gammas"]), np.asarray(inputs["betas"]))

    if l_bank not in _program_cache:
        _program_cache[l_bank] = _build_program(l_bank)
    nc = _program_cache[l_bank]

    res = run_bass_kernel_spmd(nc, in_maps, list(range(NCORES)))
    # h_out rows are in q-order; un-permute to node order
    out_q = np.concatenate(
        [res.results[c]["h_out"] for c in range(NCORES)], axis=0)
    out = np.empty_like(out_q)
    out[node_of_q] = out_q
    return out[_remap(np.arange(N_NODES))].astype(np.float32)
